# revision 49
# baseline (speedup 1.0000x reference)
"""Multi-head self-attention TRN2 Bass kernel (v2 schedule).

Problem: x[2, 2048, 1024], 16 heads x 64 dim, fp32.
Sharding: 8 cores = 2 batches x 4 head-groups (4 heads each).
Each core computes its batch's partial output; host sums 4 partials
per batch and adds bo.

v2 vs baseline: the exp stream on ACT (128 insts x ~1.11us = 142us) is
the hard floor, and PE stream work (~137us) is just below it, so the
schedule aims ACT at 100% busy from ~8us on:
  - flat software pipeline over all 128 (mt, qc, kt) score tiles with
    mt OUTER (qc inner) so deferred QKV work spreads evenly;
  - minimal prologue: only kT[m0,jc0] + qT[qc0,m0] before the first
    exp; everything else (k jc1-3, all other qT, V units, kT m1,
    out_proj) drains through a deadline-ordered filler queue popped
    by a per-slot cost budget between attention matmuls;
  - out_proj held until mt1 (outP for both pairs kept in SBUF) so it
    keeps K=256; output written bf16 (halves DMA + PSUM evacuation).
"""

import numpy as np

S = 2048          # sequence length per batch
H = 1024          # hidden
G = 256           # head-group width (4 heads x 64)
HD = 65           # V' columns per head (64 + ones)
NHL = 4           # heads per core
N_CORES = 8

_CACHE = {}


def _build():
    if "nc" in _CACHE:
        return _CACHE["nc"]

    import concourse.bass as bass
    import concourse.mybir as mybir
    import concourse.tile as tile
    from concourse import bacc
    from concourse.tile_rust import add_dep_helper

    f32 = mybir.dt.float32
    bf16 = mybir.dt.bfloat16
    EXP = mybir.ActivationFunctionType.Exp

    nc = bacc.Bacc("TRN2", target_bir_lowering=False, debug=False,
                   num_devices=N_CORES)

    # ina: per-partition [x_jc0 (4096) | wk_m0 (1024) | wq_m0 (1024)]
    # inb: per-partition [wk_m1 (1024) | wq_m1 (1024) | wo (2048)]
    ina_in = nc.dram_tensor("ina", [128, 6144], bf16, kind="ExternalInput")
    wv_in = nc.dram_tensor("wv", [128, 8, 256], bf16, kind="ExternalInput")
    inb_in = nc.dram_tensor("inb", [128, 4096], bf16, kind="ExternalInput")
    xt_in = nc.dram_tensor("xt", [4, 128, 8, 512], bf16, kind="ExternalInput")
    bq_in = nc.dram_tensor("bq", [G, 1], f32, kind="ExternalInput")
    bk_in = nc.dram_tensor("bk", [G, 1], f32, kind="ExternalInput")
    bv_in = nc.dram_tensor("bv", [G], f32, kind="ExternalInput")
    out_d = nc.dram_tensor("out", [S, H], bf16, kind="ExternalOutput")

    with tile.TileContext(nc) as tc:
        with tc.tile_pool(name="persist", bufs=1) as persist:
            qT = persist.tile([128, 2, S], bf16)     # [qd, m, s]
            kT = persist.tile([128, 2, S], bf16)
            vp = persist.tile([128, 16, NHL, HD], bf16)  # [s-part, st, h, col]
            bq_sb = persist.tile([128, 2, 1], f32)
            bk_sb = persist.tile([128, 2, 1], f32)
            bv_bc = persist.tile([128, G], f32)
            ones64 = persist.tile([1, 64], bf16)
            dum_sb = persist.tile([128, 256], bf16)
            ina = persist.tile([128, 6144], bf16)
            inb = persist.tile([128, 4096], bf16)
            wv_sb = persist.tile([128, 8, 256], bf16)
            xT1 = [persist.tile([128, 8, 512], bf16, name=f"xT_{jc}")
                   for jc in range(1, 4)]
            # views into the blobs
            xTc = [ina[:, 0:4096].rearrange("p (t s) -> p t s", t=8)] + xT1
            wk_m = [ina[:, 4096:5120].rearrange("p (t d) -> p t d", t=8),
                    inb[:, 0:1024].rearrange("p (t d) -> p t d", t=8)]
            wq_m = [ina[:, 5120:6144].rearrange("p (t d) -> p t d", t=8),
                    inb[:, 1024:2048].rearrange("p (t d) -> p t d", t=8)]
            wo_pr = inb[:, 2048:4096].rearrange("p (pr n) -> p pr n", pr=2)
            # unnormalized attention outputs, both pairs x all q-chunks
            outP = [[persist.tile([128, 512], bf16, name=f"outP_{mt}_{qc}")
                     for qc in range(4)] for mt in range(2)]

            # ---------------- DMAs ----------------
            # sync queue: x0 first (k-proj jc0 is the critical path),
            # then weights, then the remaining x chunks, all chained.
            # single sync queue, 5 big transfers in strict priority order
            # (each chain link costs ~2-4us of dead time, so pack hard):
            # blobA (x_jc0 + m0 weights + wv) -> x1 -> x2 -> x3 -> blobB
            chain = [nc.sync.dma_start(out=ina, in_=ina_in.ap())]
            for jc in range(1, 4):
                chain.append(nc.sync.dma_start(out=xT1[jc - 1],
                                               in_=xt_in.ap()[jc]))
            chain.append(nc.sync.dma_start(out=inb, in_=inb_in.ap()))
            for a, b in zip(chain, chain[1:]):
                add_dep_helper(b.ins, a.ins, reason="serialize input loads")
            # small transfers ride the gpsimd queue in parallel: the
            # warmup memset first, then wv (needed ~slot 0) + biases
            nc.gpsimd.memset(dum_sb, 0.25)
            nc.gpsimd.dma_start(out=wv_sb, in_=wv_in.ap())
            nc.gpsimd.dma_start(
                out=bq_sb, in_=bq_in.ap().rearrange("(m p) o -> p m o", p=128))
            nc.gpsimd.dma_start(
                out=bk_sb, in_=bk_in.ap().rearrange("(m p) o -> p m o", p=128))
            bv_ap = bass.AP(tensor=bv_in, offset=0, ap=[[0, 128], [1, G]])
            nc.gpsimd.dma_start(out=bv_bc, in_=bv_ap)
            nc.gpsimd.memset(vp[:, :, :, 64:65], 1.0)
            nc.gpsimd.memset(ones64, 1.0)

            # ---------------- prologue ----------------
            with tc.tile_pool(name="ps_pro", bufs=2, space="PSUM") as ps_pro:
                # HAM warm-up bridging the ~14us blobA DMA window; keeps
                # the PE at 2.4GHz so the prologue projections run warm
                for i in range(56):
                    ps_d = ps_pro.tile([128, 512], f32, tag="dum", bufs=2)
                    nc.tensor.matmul(ps_d[:, 0:256], lhsT=dum_sb[:, 0:128],
                                     rhs=dum_sb, start=True, stop=True)

                # k and q chains interleaved across two PSUM banks so
                # each LDW prefetches during the other chain's stream
                ps_k = ps_pro.tile([128, 512], f32, tag="qk", name="pro_k")
                ps_q = ps_pro.tile([128, 512], f32, tag="qk", name="pro_q")
                for ht in range(8):
                    nc.tensor.matmul(ps_k, lhsT=wk_m[0][:, ht, :],
                                     rhs=xTc[0][:, ht, :],
                                     start=(ht == 0), stop=(ht == 7))
                    nc.tensor.matmul(ps_q, lhsT=wq_m[0][:, ht, :],
                                     rhs=xTc[0][:, ht, :],
                                     start=(ht == 0), stop=(ht == 7))
                nc.vector.tensor_scalar_add(
                    kT[:, 0, 0:512], ps_k, bk_sb[:, 0, :])
                nc.vector.tensor_scalar_add(
                    qT[:, 0, 0:512], ps_q, bq_sb[:, 0, :])

            # ---------------- main pipeline ----------------
            with (
                tc.tile_pool(name="at_roll", bufs=2) as at_pool,
                tc.tile_pool(name="tmpo", bufs=4) as tmpo_pool,
                tc.tile_pool(name="sums", bufs=4) as sums_pool,
                tc.tile_pool(name="osb", bufs=2) as osb_pool,
                tc.tile_pool(name="ps_s", bufs=2, space="PSUM") as ps_s_pool,
                tc.tile_pool(name="ps_av", bufs=2, space="PSUM") as ps_av_pool,
                tc.tile_pool(name="ps_x", bufs=2, space="PSUM") as ps_x_pool,
            ):
                # ---- filler closures (each ~one slot of PE slack) ----
                # (due, cost_ns, fn): due = latest pipeline slot by which
                # the closure must be EMITTED (its consumer follows it in
                # program order); kept sorted by due.
                import bisect
                fillers = []
                _seq = [0]

                def add_filler(due, cost, fn):
                    _seq[0] += 1
                    bisect.insort(fillers, (due, _seq[0], cost, fn))

                def v_unit(st16):
                    # V' for all 4 heads, one 128-row s-tile
                    st = {}

                    def quarter(qr):
                        if qr == 0:
                            st["ps"] = ps_x_pool.tile(
                                [128, 512], f32, tag="x", name=f"psv_{st16}")
                        for ht in range(qr * 2, qr * 2 + 2):
                            nc.tensor.matmul(
                                st["ps"][:, 0:G],
                                lhsT=xTc[st16 // 4][:, ht,
                                                    (st16 % 4) * 128:
                                                    (st16 % 4 + 1) * 128],
                                rhs=wv_sb[:, ht, :],
                                start=(ht == 0), stop=(ht == 7))
                        if qr == 3:
                            nc.vector.tensor_add(
                                vp[:, st16, :, 0:64],
                                st["ps"][:, 0:G].rearrange(
                                    "p (h d) -> p h d", h=NHL),
                                bv_bc.rearrange("p (h d) -> p h d", h=NHL))

                    # consumer: av(st16), emitted during slot st16+1 (or
                    # during slot 15 itself for st16=15) -> fully popped
                    # by end of slot st16-1; 2-MM granularity so bursts
                    # never starve the score lookahead
                    for qr in range(4):
                        add_filler(max(0, st16 - 4 + qr), 240,
                                   lambda q=qr: quarter(q))

                def qk_unit(w_m, b_sb, dst, jc, m, due):
                    # one [128,512] projection chunk; four 2-matmul closures
                    st = {}

                    def quarter(qr):
                        if qr == 0:
                            st["ps"] = ps_x_pool.tile(
                                [128, 512], f32, tag="x",
                                name=f"psf_{id(w_m)}_{jc}_{m}")
                        for ht in range(qr * 2, qr * 2 + 2):
                            nc.tensor.matmul(
                                st["ps"],
                                lhsT=w_m[m][:, ht, :],
                                rhs=xTc[jc][:, ht, :],
                                start=(ht == 0), stop=(ht == 7))
                        if qr == 3:
                            nc.vector.tensor_scalar_add(
                                dst[:, m, jc * 512:(jc + 1) * 512],
                                st["ps"], b_sb[:, m, :])

                    for qr in range(4):
                        add_filler(due - (3 - qr), 450,
                                   lambda q=qr: quarter(q))

                def oproj_unit(qc, qt, due=1000, tail=False):
                    # out_proj for one q-tile (K=256: both pairs stacked)
                    osb = osb_pool.tile([128, H], bf16, tag="osb",
                                        name=f"osb_{qc}_{qt}")
                    st = {}

                    def half(ncx):
                        if tail:
                            ps_op = ps_s_pool.tile(
                                [128, 2, 512], f32, tag="s",
                                name=f"psot_{qc}_{qt}_{ncx}")[:, 0, :]
                        else:
                            ps_op = ps_x_pool.tile(
                                [128, 512], f32, tag="x",
                                name=f"pso_{qc}_{qt}_{ncx}")
                        for pr in range(2):
                            nc.tensor.matmul(
                                ps_op,
                                lhsT=outP[pr][qc][:, qt * 128:(qt + 1) * 128],
                                rhs=wo_pr[:, pr, ncx * 512:(ncx + 1) * 512],
                                start=(pr == 0), stop=(pr == 1))
                        if tail and qt % 2 == 0:
                            # ACT is idle after the last exp; split the
                            # tail PSUM evacuations across both engines
                            nc.scalar.copy(
                                osb[:, ncx * 512:(ncx + 1) * 512], ps_op)
                        else:
                            nc.vector.tensor_copy(
                                osb[:, ncx * 512:(ncx + 1) * 512], ps_op)
                        if ncx == 1:
                            nc.sync.dma_start(
                                out=out_d.ap()[qc * 512 + qt * 128:
                                               qc * 512 + (qt + 1) * 128, :],
                                in_=osb)

                    if tail:
                        half(0)
                        half(1)
                    else:
                        add_filler(due, 620, lambda: half(0))
                        add_filler(due + 1, 620, lambda: half(1))

                def norm_head_a(ps_av, qc, mt, hh, on_act=False):
                    # DVE (or ACT) part: evacuate + reciprocal chain
                    uout = tmpo_pool.tile([HD, 512], f32, tag="uout",
                                          name=f"uo_{qc}_{mt}_{hh}")
                    if on_act:
                        nc.scalar.copy(uout, ps_av)
                    else:
                        nc.vector.tensor_copy(uout, ps_av)
                    sums = sums_pool.tile([1, 512], f32, tag="sums",
                                          name=f"sm_{qc}_{mt}_{hh}")
                    nc.vector.tensor_copy(sums, uout[64:65, :])
                    recip = sums_pool.tile([1, 512], f32, tag="recip",
                                           name=f"rc_{qc}_{mt}_{hh}")
                    nc.vector.reciprocal_approx_fast(out=recip, in_=sums)
                    # partition-broadcast on the otherwise-idle gpsimd
                    # (removes 16 PE rank-1 matmuls + 16 DVE casts)
                    rbc = tmpo_pool.tile([64, 512], f32, tag="rbc",
                                         name=f"rb_{qc}_{mt}_{hh}")
                    nc.gpsimd.partition_broadcast(rbc, recip, channels=64)
                    return uout, rbc

                def norm_head_b(dst, uout, rbc, qc, mt, hh):
                    # final scale; deferred a few slots so the gpsimd
                    # broadcast latency never stalls the DVE queue.
                    # split in halves so tail out_proj can start on the
                    # first q-tiles before the second mul retires
                    for h2 in range(2):
                        sl = slice(h2 * 256, h2 * 256 + 256)
                        nc.vector.tensor_mul(
                            dst[hh * 64:hh * 64 + 64, sl], uout[0:64, sl],
                            rbc[:, sl])

                # ---- build the filler queue (sorted by due slot) ----
                for st16 in range(16):
                    v_unit(st16)
                for jc in range(1, 4):
                    # sc(j) for kt=4*jc is emitted during slot 4*jc-2,
                    # BEFORE that slot's pops -> due one slot earlier
                    qk_unit(wk_m, bk_sb, kT, jc, 0, due=4 * jc - 3)
                for qc in range(1, 4):
                    qk_unit(wq_m, bq_sb, qT, qc, 0, due=16 * qc - 3)
                # m1 work rides blocks 2-3 (blobB lands ~x3 time); mt1
                # blocks then only carry oproj fillers
                for jc in range(4):
                    qk_unit(wk_m, bk_sb, kT, jc, 1, due=26 + 4 * jc)
                for qc in range(4):
                    qk_unit(wq_m, bq_sb, qT, qc, 1, due=42 + 4 * qc)
                # oproj closures appended dynamically after mt1 norms

                # ---- the 128-slot pipeline ----
                blocks = [(mt, qc) for mt in range(2) for qc in range(4)]
                n_sc = 128

                def sc_pair(i):
                    mt, qc = blocks[i // 16]
                    kt = i % 16
                    ps_s = ps_s_pool.tile([128, 2, 512], f32, tag="s",
                                          name=f"s_{i}")
                    for hh in range(2):
                        nc.tensor.matmul(
                            ps_s[:, hh, :],
                            lhsT=kT[hh * 64:hh * 64 + 64, mt,
                                    kt * 128:(kt + 1) * 128],
                            rhs=qT[hh * 64:hh * 64 + 64, mt,
                                   qc * 512:(qc + 1) * 512],
                            start=True, stop=True)
                    return ps_s

                ps_s_tiles = {}
                attnT_tiles = {}
                ps_avs_tiles = {}

                def get_attnT(b):
                    if b not in attnT_tiles:
                        mt, qc = blocks[b]
                        attnT_tiles[b] = at_pool.tile(
                            [128, 2, 4, 512], bf16, tag="at",
                            name=f"at_{mt}_{qc}")
                        ps_avs_tiles[b] = [
                            ps_av_pool.tile([HD, 512], f32, tag="av",
                                            name=f"av_{mt}_{qc}_{hh}")
                            for hh in range(2)]
                    return attnT_tiles[b], ps_avs_tiles[b]

                def av_mms(i):
                    b = i // 16
                    mt, qc = blocks[b]
                    kt = i % 16
                    attnT, ps_avs = get_attnT(b)
                    for hh in range(2):
                        nc.tensor.matmul(
                            ps_avs[hh],
                            lhsT=vp[:, kt, 2 * mt + hh, :],
                            rhs=attnT[:, hh, kt % 4, :],
                            start=(kt == 0), stop=(kt == 15))

                def pop_fillers(i):
                    # deadline-only pops: producers must precede their
                    # consumers in program order; popping earlier would
                    # put DMA-blocked matmuls ahead of ready scores in
                    # the in-order PE queue (head-of-line blocking)
                    while fillers and fillers[0][0] <= i:
                        _, _, c, fn = fillers.pop(0)
                        fn()

                ps_s_tiles[0] = sc_pair(0)
                ps_s_tiles[1] = sc_pair(1)

                for i in range(n_sc):
                    b = i // 16
                    mt, qc = blocks[b]
                    kt = i % 16
                    # exp for slot i
                    attnT, _ = get_attnT(b)
                    nc.scalar.activation(out=attnT[:, :, kt % 4, :],
                                         in_=ps_s_tiles.pop(i), func=EXP)
                    # keep scores one group ahead
                    if i + 2 < n_sc:
                        ps_s_tiles[i + 2] = sc_pair(i + 2)
                    # attn @ V lags one slot (boundary slot already
                    # emitted its own av in the kt==15 branch below)
                    if i >= 1 and (i - 1) % 16 != 15:
                        av_mms(i - 1)
                    if kt == 15:
                        # close the block: last av + the DVE norm chains;
                        # the PE pieces (rbc+mul) become fillers so they
                        # never sit ahead of next-block scores
                        av_mms(i)
                        _, ps_avs = get_attnT(b)
                        for hh in range(2):
                            uo, rb = norm_head_a(ps_avs[hh], qc, mt, hh,
                                                 on_act=(b == 7 and hh == 0))
                            add_filler(
                                min(i + 3 + hh, 127), 300,
                                lambda u=uo, r=rb, d=outP[mt][qc], h=hh,
                                q=qc, m=mt: norm_head_b(d, u, r, q, m, h))
                        del attnT_tiles[b], ps_avs_tiles[b]
                        if mt == 1 and qc < 3:
                            # spread over the next block's slots
                            for qt in range(4):
                                oproj_unit(qc, qt, due=i + 5 + 4 * qt)
                    pop_fillers(i)

                # ---- tail: drain queue, then final out_proj with all
                # 8 PSUM banks: matmuls back-to-back, PSUM evacuations
                # split ACT/DVE, DMAs split sync/gpsimd ----
                while fillers:
                    _, _, c, fn = fillers.pop(0)
                    fn()
                tail_ps = []
                for qt in (0, 1):
                    t = ps_s_pool.tile([128, 2, 512], f32, tag="s",
                                       name=f"pst_{qt}")
                    tail_ps.append((t[:, 0, :], t[:, 1, :]))
                tail_ps.append(tuple(
                    ps_x_pool.tile([128, 512], f32, tag="x",
                                   name=f"pst_2_{ncx}") for ncx in range(2)))
                # qt3 rides the ps_s ring again (frees after qt0's copies)
                t3 = ps_s_pool.tile([128, 2, 512], f32, tag="s", name="pst_3")
                tail_ps.append((t3[:, 0, :], t3[:, 1, :]))
                for qt in range(4):
                    for ncx in range(2):
                        for pr in range(2):
                            nc.tensor.matmul(
                                tail_ps[qt][ncx],
                                lhsT=outP[pr][3][:, qt * 128:(qt + 1) * 128],
                                rhs=wo_pr[:, pr, ncx * 512:(ncx + 1) * 512],
                                start=(pr == 0), stop=(pr == 1))
                osbs = []
                for qt in range(4):
                    osb = osb_pool.tile([128, H], bf16, tag="osbt",
                                        name=f"osbt_{qt}", bufs=4)
                    osbs.append(osb)
                    for ncx in range(2):
                        dst = osb[:, ncx * 512:(ncx + 1) * 512]
                        if (2 * qt + ncx) % 2 == 0:
                            nc.scalar.copy(dst, tail_ps[qt][ncx])
                        else:
                            nc.vector.tensor_copy(dst, tail_ps[qt][ncx])
                for qt in range(4):
                    nc.sync.dma_start(
                        out=out_d.ap()[3 * 512 + qt * 128:
                                       3 * 512 + (qt + 1) * 128, :],
                        in_=osbs[qt])

    nc.compile()
    _CACHE["nc"] = nc
    return nc


def make_in_maps(x, Wq, bq, Wk, bk, Wv, bv, Wo):
    import ml_dtypes
    bf = ml_dtypes.bfloat16

    x = np.asarray(x, dtype=np.float32)
    Wq = np.asarray(Wq, dtype=np.float32)
    bq = np.asarray(bq, dtype=np.float32)
    Wk = np.asarray(Wk, dtype=np.float32)
    bk = np.asarray(bk, dtype=np.float32)
    Wv = np.asarray(Wv, dtype=np.float32)
    bv = np.asarray(bv, dtype=np.float32)
    Wo = np.asarray(Wo, dtype=np.float32)

    scale = np.float32(1.0 / 8.0)  # 1/sqrt(64)

    in_maps = []
    for core in range(N_CORES):
        b = core // 4
        g = core % 4
        cs = slice(g * G, (g + 1) * G)
        def msplit(w):  # [1024, 256] -> [2(m), 128(p), 8(t)*128(d)]
            return w.reshape(8, 128, 2, 128).transpose(2, 1, 0, 3).reshape(
                2, 128, 1024)

        xt = x[b].reshape(4, 512, 8, 128).transpose(0, 3, 2, 1)  # [jc,p,t,s]
        wqm = msplit(Wq[:, cs] * scale)
        wkm = msplit(Wk[:, cs])
        wvp = Wv[:, cs].reshape(8, 128, 256).transpose(1, 0, 2).reshape(
            128, 2048)  # [p, t*d]
        # wo rows (two*64+p) x [pr] x H, matching lhsT=outP[pr] partitions
        wo4 = Wo[cs, :].reshape(2, 2, 64, H)          # [pr, two, p64, H]
        wop = np.ascontiguousarray(wo4.transpose(1, 2, 0, 3)).reshape(
            128, 2, H)
        ina = np.concatenate([
            xt[0].reshape(128, 4096), wkm[0], wqm[0]], axis=1)
        inb = np.concatenate([wkm[1], wqm[1],
                              wop.reshape(128, 2 * H)], axis=1)
        in_maps.append({
            "ina": np.ascontiguousarray(ina).astype(bf),
            "wv": np.ascontiguousarray(wvp.reshape(128, 8, 256)).astype(bf),
            "inb": np.ascontiguousarray(inb).astype(bf),
            "xt": np.ascontiguousarray(xt).astype(bf),
            "bq": np.ascontiguousarray((bq[cs] * scale).reshape(G, 1)),
            "bk": np.ascontiguousarray(bk[cs].reshape(G, 1)),
            "bv": np.ascontiguousarray(bv[cs]),
        })
    return in_maps


def kernel(x, Wq, bq, Wk, bk, Wv, bv, Wo, bo):
    from concourse.bass_utils import run_bass_kernel_spmd

    bo = np.asarray(bo, dtype=np.float32)
    nc = _build()
    in_maps = make_in_maps(x, Wq, bq, Wk, bk, Wv, bv, Wo)
    res = run_bass_kernel_spmd(nc, in_maps, core_ids=list(range(N_CORES)))

    out = np.empty((2, S, H), dtype=np.float32)
    for b in range(2):
        acc = res.results[4 * b]["out"].astype(np.float32)
        for g in range(1, 4):
            acc = acc + res.results[4 * b + g]["out"].astype(np.float32)
        out[b] = acc + bo
    return out
